# revision 46
# baseline (speedup 1.0000x reference)
"""Trainium2 8-core kernel for a single-head AttentionBlock.

Reference computation (fp32, per batch b):
    qkv = x @ w_qkv.T + b_qkv            # [S, 3H]
    q, k, v = split(qkv)                 # each [S, H]
    scores = q @ k.T / sqrt(H)           # [S, S]
    probs = softmax(scores, -1)
    ctx = probs @ v                      # [S, H]
    out = ctx @ w_out.T + b_out          # [S, H]

Shapes: B=4, S=2048, H=2048 (single head, head_dim = H).

Algebraic fold (exact): with A = Wq^T Wk / sqrt(H),
    scores = x_q A x^T (+ per-row consts that drop under softmax)
             (+ u.x per-key term, u = bq Wk / sqrt(H), folded into exp bias)
    ctx @ Wo^T = (P x) (Wo Wv)^T + (P 1)(Wo bv)^T, and P 1 = 1 after
    normalization, so out = (P x) Weq2^T + (Wo bv + bo).
This removes the K and V projections entirely (4 big matmul stages/core
instead of 6) and needs no collectives: every core just gets its batch's
x in two layouts. A := folded on the host (weight-only precompute).

Sharding: 8 cores = 4 batches x 2 query-halves. Core c handles batch
b = c // 2 and query half qc = c % 2. x is permuted per core to local
order (own half first) so the SPMD graph is identical on all cores.

Compute is bf16 on the TensorEngine with fp32 PSUM accumulation; softmax
runs in fp32 (exp on ScalarE).
"""

import math

import numpy as np
import ml_dtypes

import concourse.bacc as bacc
import concourse.tile as tile
import concourse.mybir as mybir
from concourse import bass_isa
from concourse.bass_utils import run_bass_kernel_spmd

BF16 = ml_dtypes.bfloat16
F32 = mybir.dt.float32
BF = mybir.dt.bfloat16

B, S, H = 4, 2048, 2048
SQ = S // 2          # queries per core
HT = H // 128        # 16 h-chunks
N_CORES = 8


def build_graph():
    nc = bacc.Bacc(
        "TRN2", target_bir_lowering=False, debug=False, num_devices=N_CORES
    )

    # ---- DRAM parameters (per-core shards, host-prepared layouts) ----
    # xt[p, ht, s] = x_loc[s, ht*128+p]  (x transposed, local seq order:
    # own query half first; serves as stage-1 rhs AND as score keys)
    xt_e = nc.dram_tensor("xt", [128, HT, S], BF, kind="ExternalInput")
    # xs[st, p, h] = x_loc[st*128+p, h]  (row-major seq chunks for P@x)
    xs_e = nc.dram_tensor("xs", [S // 128, 128, H], BF, kind="ExternalInput")
    # m1[ot, p, ht, m] = Weq1[ot*128+m, ht*128+p], Weq1 = Wk^T Wq / sqrt(H)
    m1_e = nc.dram_tensor("m1", [16, 128, HT, 128], BF, kind="ExternalInput")
    # m2[ob, p, ht, n] = Weq2[ob*512+n, ht*128+p], Weq2 = Wo @ Wv
    m2_e = nc.dram_tensor("m2", [4, 128, HT, 512], BF, kind="ExternalInput")
    # ux[p, c] = (bq Wk / sqrt(H)) . x_loc[c*128+p]  (per-key exp bias)
    ux_e = nc.dram_tensor("ux", [128, 16], F32, kind="ExternalInput")

    out_e = nc.dram_tensor("out", [SQ, H], F32, kind="ExternalOutput")

    with tile.TileContext(nc) as tc:
        with (
            tc.tile_pool(name="const", bufs=1) as cpool,
            tc.tile_pool(name="small", bufs=1) as spool,
            tc.tile_pool(name="psum", bufs=8, space="PSUM") as pp,
        ):
            # persistent tiles
            s1T = cpool.tile([128, HT, SQ], BF, tag="s1T")     # 32KB/p
            tT = cpool.tile([128, HT, SQ], BF, tag="tT")       # 32KB/p
            # x_loc^T for the q half: stage-1 rhs, reused as the first
            # two key slabs of the scores stage (keys 0..1023). Split in
            # ht-quarters (separate tiles) so the first matmuls only wait
            # on the first quarter's DMA.
            # sb0's first quarter is split per-ht so the very first matmul
            # only waits on a 1KB/p DMA
            xq0s = [
                cpool.tile([128, 1, 512], BF, tag=f"xq0s{i}", name=f"xq0s{i}")
                for i in range(4)
            ]
            xq = [None] + [
                cpool.tile([128, 4, 512], BF, tag=f"xq{i}", name=f"xq{i}")
                for i in range(1, 8)
            ]                                                  # 4KB/p each
            ux = cpool.tile([128, 16], F32, tag="ux")

            def xq_sl(sb, ht, k_sl=slice(None)):
                if sb == 0 and ht < 4:
                    return xq0s[ht][:, 0, k_sl]
                return xq[sb * 4 + ht // 4][:, ht % 4, k_sl]

            def xq_ap(sb, ht):
                return xq_sl(sb, ht)

            # ================= Phase P: s1 = x_q @ M1 =================
            with tc.tile_pool(name="m1", bufs=10) as m1p:
                # weight slabs in ht-halves; the first half-slab DMA goes
                # ahead of the x bulk so the first matmul group's
                # stationary operand isn't queued behind it
                def m1_tiles(pre=False):
                    base = "w_pre" if pre else "m1w"
                    return [
                        m1p.tile([128, 8, 128], BF, tag="m1w", name=f"{base}_{j}")
                        for j in range(2)
                    ]

                def m1_load(ts, ot):
                    for j in range(2):
                        nc.sync.dma_start(
                            out=ts[j][:], in_=m1_e[ot, :, j * 8 : (j + 1) * 8, :]
                        )

                # sb-outer: all 16 weight groups run on query-block 0
                # first, so only sb0's 16KB/p of x is startup-critical;
                # sb1's x and the (re-streamed) weight slabs arrive far
                # ahead of their use. m1 is read twice — DMA is cheap here.
                # startup-critical DMA stream, ordered by first use
                w_pre = m1_tiles(pre=True)
                nc.sync.dma_start(out=w_pre[0][:], in_=m1_e[0, :, 0:8, :])
                nc.sync.dma_start(out=xq0s[0][:], in_=xt_e[:, 0:1, 0:512])
                nc.sync.dma_start(out=xq0s[1][:], in_=xt_e[:, 1:2, 0:512])
                nc.sync.dma_start(out=w_pre[1][:], in_=m1_e[0, :, 8:16, :])
                nc.sync.dma_start(out=xq0s[2][:], in_=xt_e[:, 2:3, 0:512])
                nc.sync.dma_start(out=xq0s[3][:], in_=xt_e[:, 3:4, 0:512])
                for i in range(1, 4):
                    nc.sync.dma_start(
                        out=xq[i][:],
                        in_=xt_e[:, (i % 4) * 4 : (i % 4) * 4 + 4, 0:512],
                    )
                # ux (128 tiny descriptors) issued after the
                # startup-critical stream; it isn't read until phase A
                nc.sync.dma_start(out=ux[:], in_=ux_e.ap())

                for sb in range(2):
                    for ot in range(16):
                        if sb == 0 and ot == 0:
                            w = w_pre
                        else:
                            w = m1_tiles()
                            m1_load(w, ot)
                        if sb == 0 and ot == 4:
                            # sb1's x, needed ~45us from now
                            for i in range(4, 8):
                                nc.sync.dma_start(
                                    out=xq[i][:],
                                    in_=xt_e[:, (i % 4) * 4 : (i % 4) * 4 + 4,
                                             512:1024],
                                )
                        ps = pp.tile([128, 512], F32, tag="ps")
                        for ht in range(HT):
                            nc.tensor.matmul(
                                ps[:],
                                w[ht // 8][:, ht % 8, :],
                                xq_ap(sb, ht),
                                start=(ht == 0),
                                stop=(ht == HT - 1),
                            )
                        nc.scalar.activation(
                            s1T[:, ot, sb * 512 : (sb + 1) * 512],
                            ps[:],
                            mybir.ActivationFunctionType.Identity,
                        )

            # ================= Phase A: attention + out proj =========
            with (
                tc.tile_pool(name="ks", bufs=2) as kp,
                tc.tile_pool(name="probs", bufs=1) as prp,
                tc.tile_pool(name="vs", bufs=6) as vp,
                tc.tile_pool(name="m2", bufs=2) as wop,
                tc.tile_pool(name="ost", bufs=3) as op,
            ):
                # prefetch the first out-proj weight slab during attention
                w_m2_0 = wop.tile([128, HT, 512], BF, tag="m2w", name="m2pre")
                nc.sync.dma_start(out=w_m2_0[:], in_=m2_e[0])
                for qb in range(2):
                    q_sl = slice(qb * 512, (qb + 1) * 512)
                    probs = prp.tile([128, 16, 512], BF, tag="probs")
                    den = spool.tile([128, 512], F32, tag="den")
                    # ---- scores + exp; key slabs 0,1 are the resident
                    # q-half tiles, slabs 2,3 stream from DRAM ----
                    for skg in range(4):
                        ks = None
                        if skg >= 2:
                            ks = kp.tile([128, HT, 512], BF, tag="ks")
                            nc.sync.dma_start(
                                out=ks[:],
                                in_=xt_e[:, :, skg * 512 : (skg + 1) * 512],
                            )
                        for skw in range(4):
                            sk = skg * 4 + skw
                            k_sl = slice(skw * 128, (skw + 1) * 128)
                            ps = pp.tile([128, 512], F32, tag="ps")
                            for ht in range(HT):
                                nc.tensor.matmul(
                                    ps[:],
                                    ks[:, ht, k_sl]
                                    if ks is not None
                                    else xq_sl(skg, ht, k_sl),
                                    s1T[:, ht, q_sl],
                                    start=(ht == 0),
                                    stop=(ht == HT - 1),
                                )
                            nc.scalar.activation(
                                probs[:, sk, :],
                                ps[:],
                                mybir.ActivationFunctionType.Exp,
                                bias=ux[:, sk : sk + 1],
                            )
                            if sk == 0:
                                nc.vector.tensor_copy(den[:], probs[:, 0, :])
                            else:
                                nc.vector.tensor_add(
                                    den[:], den[:], probs[:, sk, :]
                                )
                    # ---- t^T accumulation (t = P @ x), two passes of 8
                    # h-chunks (all 8 PSUM banks). vs tiles span 1024
                    # h-cols and are pre-issued 4 ahead: one DMA feeds 8
                    # matmuls, so the ~0.6us/DMA Sync issue rate stays
                    # well ahead of the PE. ----
                    def ctx_pass(hp, mid=None):
                        cps = [
                            pp.tile([128, 512], F32, tag="ps", name=f"cps{i}")
                            for i in range(8)
                        ]
                        vss = {}

                        def issue_vs(sk):
                            t = vp.tile([128, 1024], BF, tag="vs")
                            nc.sync.dma_start(
                                out=t[:],
                                in_=xs_e[sk, :, hp * 1024 : (hp + 1) * 1024],
                            )
                            vss[sk] = t

                        for sk in range(4):
                            issue_vs(sk)
                        for sk in range(16):
                            if sk + 4 < 16:
                                issue_vs(sk + 4)
                            vs = vss.pop(sk)
                            for hl in range(8):
                                nc.tensor.matmul(
                                    cps[hl][:],
                                    vs[:, hl * 128 : (hl + 1) * 128],
                                    probs[:, sk, :],
                                    start=(sk == 0),
                                    stop=(sk == 15),
                                )
                            if sk == 1 and mid is not None:
                                mid()
                        return cps

                    def ctx_norm(hp, cps):
                        for hl in range(8):
                            nc.vector.tensor_mul(
                                tT[:, hp * 8 + hl, q_sl], cps[hl][:], rb[:]
                            )

                    # Denominator all-reduce runs on the (otherwise idle)
                    # GpSimd engine, injected into hg 0's matmul stream
                    # after sk 1 — the PE never touches the softmax
                    # denominator at all.
                    denall = spool.tile([128, 512], F32, tag="denall")
                    rb = spool.tile([128, 512], F32, tag="rb")

                    def mid_den():
                        nc.gpsimd.partition_all_reduce(
                            denall[:], den[:], channels=128,
                            reduce_op=bass_isa.ReduceOp.add,
                        )
                        # ~51-ULP approx is plenty for bf16 outputs and 5x
                        # faster; den is strictly positive and well-scaled
                        nc.vector.reciprocal_approx_fast(rb[:], denall[:])

                    cpsA = ctx_pass(0, mid=mid_den)
                    ctx_norm(0, cpsA)
                    ctx_norm(1, ctx_pass(1))
                # ---- output projection: out = t @ M2 (the +c bias is a
                # per-column constant, added exactly on the host during
                # unshard). PSUM drains via a ScalarE copy — cheaper than
                # the DVE add and ScalarE is idle here. ----
                for ob in range(4):
                    if ob == 0:
                        w = w_m2_0
                    else:
                        w = wop.tile([128, HT, 512], BF, tag="m2w")
                        nc.sync.dma_start(out=w[:], in_=m2_e[ob])
                    for st in range(SQ // 128):
                        ps = pp.tile([128, 512], F32, tag="ps")
                        for ht in range(HT):
                            nc.tensor.matmul(
                                ps[:],
                                tT[:, ht, st * 128 : (st + 1) * 128],
                                w[:, ht, :],
                                start=(ht == 0),
                                stop=(ht == HT - 1),
                            )
                        ost = op.tile([128, 512], F32, tag="ost")
                        nc.vector.tensor_copy(ost[:], ps[:])
                        nc.sync.dma_start(
                            out=out_e[st * 128 : (st + 1) * 128,
                                      ob * 512 : (ob + 1) * 512],
                            in_=ost[:],
                        )

    nc.compile()
    return nc


def prep_inputs(hidden_states, w_qkv, b_qkv, w_out, b_out):
    """Build the 8 per-core input maps (host-side fold + layout)."""
    hidden_states = np.asarray(hidden_states, dtype=np.float32)
    w_qkv = np.asarray(w_qkv, dtype=np.float32)
    b_qkv = np.asarray(b_qkv, dtype=np.float32)
    w_out = np.asarray(w_out, dtype=np.float32)
    b_out = np.asarray(b_out, dtype=np.float32)

    scale = 1.0 / math.sqrt(H)
    wq = w_qkv[:H]
    wk = w_qkv[H : 2 * H]
    wv = w_qkv[2 * H :]
    bq = b_qkv[:H]
    bv = b_qkv[2 * H :]

    # weight-only folds (host precompute, input-independent)
    weq1 = (wk.T @ wq) * scale          # [H, H]: s1 = x_q @ weq1^T
    weq2 = w_out @ wv                   # [H, H]: out = t @ weq2^T
    u = (bq @ wk) * scale               # [H]
    c = w_out @ bv + b_out              # [H]

    # m1[ot, p, ht, m] = weq1[ot*128+m, ht*128+p]
    m1_l = np.ascontiguousarray(
        weq1.reshape(16, 128, HT, 128).transpose(0, 3, 2, 1)
    ).astype(BF16)
    # m2[ob, p, ht, n] = weq2[ob*512+n, ht*128+p]
    m2_l = np.ascontiguousarray(
        weq2.reshape(4, 512, HT, 128).transpose(0, 3, 2, 1)
    ).astype(BF16)

    in_maps = []
    for core in range(N_CORES):
        b, qc = divmod(core, 2)
        x = hidden_states[b]  # [S, H]
        if qc == 1:
            # local sequence order: own half first
            x = np.concatenate([x[SQ:], x[:SQ]], axis=0)
        xbf = x.astype(BF16)
        # xt[p, ht, s] = x[s, ht*128+p]
        xt = np.ascontiguousarray(
            xbf.T.reshape(HT, 128, S).transpose(1, 0, 2)
        )
        # xs[st, p, h] = x[st*128+p, h]
        xs = np.ascontiguousarray(xbf.reshape(S // 128, 128, H))
        ux_full = x @ u  # [S] in local key order
        ux_l = np.ascontiguousarray(
            ux_full.reshape(16, 128).T
        ).astype(np.float32)
        in_maps.append(
            {
                "xt": xt,
                "xs": xs,
                "m1": m1_l,
                "m2": m2_l,
                "ux": ux_l,
            }
        )
    return in_maps, c


_CACHED = {}


def _get_graph():
    if "g" not in _CACHED:
        _CACHED["g"] = build_graph()
    return _CACHED["g"]


def run(hidden_states, w_qkv, b_qkv, w_out, b_out, trace=False):
    nc = _get_graph()
    in_maps, c = prep_inputs(hidden_states, w_qkv, b_qkv, w_out, b_out)
    res = run_bass_kernel_spmd(
        nc, in_maps, list(range(N_CORES)), trace=trace
    )
    out = np.empty((B, S, H), dtype=np.float32)
    for core in range(N_CORES):
        b, qc = divmod(core, 2)
        out[b, qc * SQ : (qc + 1) * SQ] = res.results[core]["out"]
    out += c  # per-column constant Wo@bv + bo, exact in f32
    return out, res


def kernel(hidden_states, w_qkv, b_qkv, w_out, b_out):
    out, _ = run(hidden_states, w_qkv, b_qkv, w_out, b_out)
    return out


if __name__ == "__main__":
    rng = np.random.default_rng(0)
    hs = rng.standard_normal((B, S, H)).astype(np.float32)
    a1 = math.sqrt(6.0 / (H + 3 * H))
    a2 = math.sqrt(6.0 / (2 * H))
    wq = rng.uniform(-a1, a1, (3 * H, H)).astype(np.float32)
    wo = rng.uniform(-a2, a2, (H, H)).astype(np.float32)
    out = kernel(hs, wq, np.zeros(3 * H, np.float32), wo, np.zeros(H, np.float32))
    print(out.shape, out.dtype)
